# revision 47
# baseline (speedup 1.0000x reference)
"""Trainium2 Bass kernel for nn_CausalSelfAttention (B=2, T=2048, C=1024, 16 heads).

Sharding: 8 cores = 2 batches x 4 head-groups (4 heads each).

v3 design (cost-model driven):
  - QKV projections run as fp8(e4m3) DoubleRow matmuls with hi/lo error
    compensation: 32*x@W = xh@fp8(32W) + fp8(32(x-xh))@fp8(W)
    + xh@fp8(32W - fp8(32W)), all three terms accumulated in one PSUM
    group. DR charges 0.5 cycles/output-col and packs 2 contraction
    chunks per instruction, so this costs 0.75x of bf16 at ~0.3% error
    (plain fp8 would be 0.25x cost but ~5% error -- softmax averaging
    shrinks signal and noise equally, so that error hits the output
    full-strength and blows the 2e-2 gate).
  - S = Q^T K, P (exp output), P@V, and the out-projection stay bf16.
  - exp runs on ACT (~73us busy); PE (~90us busy) is the bottleneck, the
    emission weaves S tiles and filler work by a cost ledger.
  - diagonal causal masking: post-exp multiply by triu-ones on the
    (otherwise idle) gpsimd/Pool engine.
  - O is normalized in [q,d] layout then transposed via PE (identity
    matmul) + DVE copy -- no DMA XBAR on the critical path.
  - out-projection in bf16, partial [T,C] per core; host sums the 4
    tensor-parallel partials per batch and adds b_out.
"""
import sys

if '/opt/trn_rl_repo' not in sys.path:
    sys.path.insert(0, '/opt/trn_rl_repo')

import numpy as np
import ml_dtypes

B, T, C = 2, 2048, 1024
N_HEAD = 16
D = 64
P = 128
N_CORES = 8
GROUPS = N_CORES // B            # 4 tensor-parallel groups per batch
HPC = N_HEAD // GROUPS           # 4 heads per core
DH = HPC * D                     # 256 head dims per core
KO = C // P                      # 8 contraction subtiles for projections
NQB = T // 512                   # 4 q blocks of 512
WSCALE = 32.0                    # fp8 range scaling of W_qkv
SCALE = 1.0 / (np.sqrt(D) * WSCALE * WSCALE)   # exp scale

_CACHE = {}

# weave pacing costs (ns)
PE_CYC = 0.4167


def _build():
    import concourse.mybir as mybir
    import concourse.tile as tile
    from concourse import bacc

    f32 = mybir.dt.float32
    bf16 = mybir.dt.bfloat16
    f16 = mybir.dt.float16
    fp8 = mybir.dt.float8e4
    DR = mybir.MatmulPerfMode.DoubleRow
    EXP = mybir.ActivationFunctionType.Exp
    MUL = mybir.AluOpType.mult
    ADD = mybir.AluOpType.add

    nc = bacc.Bacc("TRN2", target_bir_lowering=False, debug=False,
                   num_devices=N_CORES)

    xh_d = nc.dram_tensor("xh", [C, T], fp8, kind="ExternalInput")
    xl_d = nc.dram_tensor("xl", [C, T], fp8, kind="ExternalInput")
    w_ds = {}
    for nm in ("wqh", "wq1", "wql", "wkh", "wk1", "wkl",
               "wvh", "wv1", "wvl"):
        w_ds[nm] = nc.dram_tensor(nm, [P, 2, KO, P], fp8,
                                  kind="ExternalInput")
    wo_d = nc.dram_tensor("wo", [P, 2, C], bf16, kind="ExternalInput")
    bq_d = nc.dram_tensor("bq", [P, 2], f32, kind="ExternalInput")
    bk_d = nc.dram_tensor("bk", [P, 2], f32, kind="ExternalInput")
    bv_d = nc.dram_tensor("bv", [1, DH], f32, kind="ExternalInput")
    tri_d = nc.dram_tensor("tri", [P, 2, P], bf16, kind="ExternalInput")
    id_d = nc.dram_tensor("ident", [P, P], bf16, kind="ExternalInput")
    out_d = nc.dram_tensor("out", [T, C], f16, kind="ExternalOutput")

    pls = [slice(0, D), slice(D, 2 * D)]

    with tile.TileContext(nc) as tc:
        with (
            tc.tile_pool(name="pp", bufs=1) as pp,
            tc.tile_pool(name="wS", bufs=4) as wS,
            tc.tile_pool(name="wO", bufs=4) as wO,
            tc.tile_pool(name="psS", bufs=2, space="PSUM") as psS,
            tc.tile_pool(name="psU", bufs=4, space="PSUM") as psU,
        ):
            xhs_t = [pp.tile([P, KO, 512], fp8, tag=f"xh{q}", name=f"xh{q}")
                     for q in range(4)]
            xls_t = [pp.tile([P, KO, 512], fp8, tag=f"xl{q}", name=f"xl{q}")
                     for q in range(4)]

            def x_mv(xt_, q, t):
                # moving operand [p, 2(ko-pair), 512]
                return xt_[q][:, 2 * t:2 * t + 2, :]

            def x_st(xt_, q, t, it_):
                # stationary operand [p, 2(ko-pair), 128]
                return xt_[q][:, 2 * t:2 * t + 2, it_ * P:(it_ + 1) * P]
            wt = {nm: pp.tile([P, 2, KO, P], fp8, tag=nm, name=nm)
                  for nm in ("wqh", "wq1", "wql", "wkh", "wk1", "wkl",
                             "wvh", "wv1", "wvl")}
            wos = pp.tile([P, 2, C], bf16, tag="wos")
            qts = [[pp.tile([P, 512], bf16, tag=f"qt{s}_{q}", name=f"qt{s}_{q}")
                    for q in range(4)] for s in range(2)]
            kts = [[pp.tile([P, 512], bf16, tag=f"kt{s}_{q}", name=f"kt{s}_{q}")
                    for q in range(4)] for s in range(2)]
            # vo: [k-part, kt-slot, head(2hs+u), d + ones-col]
            vo = pp.tile([P, 16, HPC, D + 1], bf16, tag="vo")
            # pt: per hs P^T tiles [k-part, kt-slot, u, q-col of current block]
            pts = [pp.tile([P, 16, 2, 512], bf16, tag=f"pt{hs}", name=f"pt{hs}")
                   for hs in range(2)]
            ots = [[pp.tile([P, 512], bf16, tag=f"ot{j}_{hs}",
                            name=f"ot{j}_{hs}") for hs in range(2)]
                   for j in range(NQB)]
            bqs = pp.tile([P, 2], f32, tag="bqs")
            bks = pp.tile([P, 2], f32, tag="bks")
            bvrow = pp.tile([1, DH], f32, tag="bvrow")
            bvb = pp.tile([P, DH], f32, tag="bvb")
            trib = pp.tile([P, 2, P], bf16, tag="trib")
            ident = pp.tile([P, P], bf16, tag="ident")
            scr = pp.tile([1, 1], f32, tag="scr")

            # trigger the exp ACT-table load immediately (scratch memset)
            nc.vector.memset(scr[:], 0.0)
            nc.scalar.activation(scr[0:1, 0:1], scr[0:1, 0:1], EXP)
            # ones-column (=WSCALE) of V for softmax denominators
            nc.vector.memset(vo[:, :, :, D:D + 1], WSCALE)

            # ---- loads (sync=SP queue, scalar=ACT queue, gpsimd=SWDGE) ----
            xh_r = xh_d.rearrange("(ko p) t -> p ko t", p=P)
            xl_r = xl_d.rearrange("(ko p) t -> p ko t", p=P)
            # critical chain (first S tile): wqh,xh0,xl0,wq1,wql + wk* on
            # SWDGE. Big non-critical loads go late; sync gets tiny loads
            # so the round-robin HWDGE issue doesn't let x quarters jump
            # ahead of critical weight loads.
            for nm in ("wqh", "wq1", "wql"):
                nc.sync.dma_start(wt[nm][:, 0], w_ds[nm][:, 0])
            nc.scalar.dma_start(xhs_t[0][:], xh_r[:, :, 0:512])
            nc.scalar.dma_start(xls_t[0][:], xl_r[:, :, 0:512])
            for nm in ("wkh", "wk1", "wkl"):
                nc.gpsimd.dma_start(wt[nm][:, 0], w_ds[nm][:, 0])
            nc.sync.dma_start(bqs[:], bq_d[:])
            nc.sync.dma_start(bks[:], bk_d[:])
            for nm in ("wqh", "wq1", "wql"):
                nc.sync.dma_start(wt[nm][:, 1], w_ds[nm][:, 1])
            for nm in ("wkh", "wk1", "wkl"):
                nc.gpsimd.dma_start(wt[nm][:, 1], w_ds[nm][:, 1])
            nc.sync.dma_start(bvrow[:], bv_d[:])
            nc.sync.dma_start(trib[:], tri_d[:])
            nc.sync.dma_start(ident[:], id_d[:])
            for nm in ("wvh", "wv1", "wvl"):
                nc.sync.dma_start(wt[nm][:], w_ds[nm][:])
            for q in range(1, 4):
                nc.scalar.dma_start(xhs_t[q][:],
                                    xh_r[:, :, q * 512:(q + 1) * 512])
                nc.scalar.dma_start(xls_t[q][:],
                                    xl_r[:, :, q * 512:(q + 1) * 512])
            nc.sync.dma_start(wos[:], wo_d[:])

            nc.gpsimd.partition_broadcast(bvb[:, :], bvrow[0:1, :])

            # ---------- item constructors (thunk, pe_ns, act_ns) ----------
            # hi/lo fp8 compensation: 32xW = xh@Wh + xl@W1 + xh@Wl, all
            # accumulated in one PSUM group (12 DR steps).
            def qk8_item(q, wpfx, dsts, bias, s_):
                terms = [(wt[wpfx + "h"], xhs_t), (wt[wpfx + "1"], xls_t),
                         (wt[wpfx + "l"], xhs_t)]

                def it():
                    pq = psU.tile([P, 512], f32, tag="ps1",
                                  name=f"pq{q}_{wpfx}_{s_}")
                    for ti, (w8, xsrc) in enumerate(terms):
                        for t in range(KO // 2):
                            nc.tensor.matmul(
                                pq[:],
                                w8[:, s_, 2 * t:2 * t + 2, :],
                                x_mv(xsrc, q, t),
                                start=(ti == 0 and t == 0),
                                stop=(ti == 2 and t == KO // 2 - 1),
                                perf_mode=DR)
                    nc.vector.tensor_scalar_add(
                        dsts[s_][q][:], pq[:], bias[:, s_:s_ + 1])
                return (it, 1290, 0)

            def qk8_items(q):
                # order: Q s0, K s0, Q s1, K s1 (heads-split 0 first so the
                # next phase's S(q,0) stream unblocks earliest)
                return [qk8_item(q, "wq", qts, bqs, 0),
                        qk8_item(q, "wk", kts, bks, 0),
                        qk8_item(q, "wq", qts, bqs, 1),
                        qk8_item(q, "wk", kts, bks, 1)]

            def v8_item(q, it_):
                terms = [(wt["wvh"], xhs_t), (wt["wv1"], xls_t),
                         (wt["wvl"], xhs_t)]

                def it():
                    pv = psU.tile([P, DH], f32, tag="ps1",
                                  name=f"pv{q}_{it_}")
                    for ti, (w8, xsrc) in enumerate(terms):
                        for t in range(KO // 2):
                            nc.tensor.matmul(
                                pv[:],
                                x_st(xsrc, q, t, it_),
                                w8[:, :, 2 * t:2 * t + 2, :].rearrange(
                                    "p s k d -> p k s d"),
                                start=(ti == 0 and t == 0),
                                stop=(ti == 2 and t == KO // 2 - 1),
                                perf_mode=DR)
                    nc.vector.tensor_tensor(
                        vo[:, 4 * q + it_, :, 0:D],
                        pv[:].rearrange("p (h d) -> p h d", h=HPC),
                        bvb.rearrange("p (h d) -> p h d", h=HPC),
                        ADD)
                return (it, 645, 0)

            def s_item(q, hs, i):
                off = max(0, P * i - 512 * q)
                diag = P * i >= 512 * q

                def it():
                    sp = psS.tile([P, 2, 512], f32, tag="sp",
                                  name=f"sp{q}_{hs}_{i}")
                    for u in range(2):
                        nc.tensor.matmul(
                            sp[:, u, off:512],
                            kts[hs][i // 4][pls[u],
                                            (i % 4) * P:(i % 4 + 1) * P],
                            qts[hs][q][pls[u], off:512],
                            start=True, stop=True)
                    nc.scalar.activation(pts[hs][:, i, :, off:512],
                                         sp[:, :, off:512],
                                         EXP, scale=float(SCALE))
                    if diag:
                        nc.gpsimd.tensor_tensor(
                            pts[hs][:, i, :, off:off + P],
                            pts[hs][:, i, :, off:off + P],
                            trib[:], MUL)
                w = 512 - off
                return (it, 2 * w * PE_CYC + 20, 2 * w * 0.833 + 190)

            def pv_item(q, hs, qc, u):
                qt = 4 * q + qc

                def it():
                    po = psU.tile([P, 512], f32, tag="ps1",
                                  name=f"po{q}_{hs}_{qc}_{u}")
                    for kt in range(qt + 1):
                        nc.tensor.matmul(
                            po[:, 0:D + 1],
                            pts[hs][:, kt, u, qc * P:(qc + 1) * P],
                            vo[:, kt, 2 * hs + u, :],
                            start=(kt == 0), stop=(kt == qt))
                    rd = wS.tile([P, 1], f32, tag="rd",
                                 name=f"rd{q}_{hs}_{qc}_{u}")
                    if u == 0:
                        osbs[(q, hs, qc)] = wS.tile(
                            [P, 2, D], bf16, tag="osb",
                            name=f"osb{q}_{hs}_{qc}")
                    osb = osbs[(q, hs, qc)]
                    nc.vector.reciprocal_approx_fast(rd[:], po[:, D:D + 1])
                    nc.vector.tensor_scalar_mul(
                        osb[:, u, :], po[:, 0:D], rd[:])
                return (it, (qt + 1) * 27 + 30, 0)

            osbs = {}

            def tr_item(q, hs, qc):
                def it():
                    ptr = psU.tile([P, P], bf16, tag="ps1",
                                   name=f"ptr{q}_{hs}_{qc}")
                    nc.tensor.transpose(
                        ptr[:], osbs[(q, hs, qc)].rearrange("p u d -> p (u d)"),
                        ident[:])
                    nc.vector.tensor_copy(
                        ots[q][hs][:, qc * P:(qc + 1) * P], ptr[:])
                return (it, 75, 0)

            def c_item(jj, mo, n, tail=False):
                def it():
                    pc = psU.tile([P, 512], f32, tag="ps1",
                                  name=f"pc{jj}_{mo}_{n}")
                    for s in range(2):
                        nc.tensor.matmul(
                            pc[:],
                            ots[jj][s][:, mo * P:(mo + 1) * P],
                            wos[:, s, n * 512:(n + 1) * 512],
                            start=(s == 0), stop=(s == 1))
                    if n == 0:
                        obs[(jj, mo)] = wO.tile([P, C], f16, tag="ob",
                                                name=f"ob{jj}_{mo}")
                    ob = obs[(jj, mo)]
                    if tail:
                        # ACT is idle after the last exp; offload the copy
                        # so DVE can serve the critical osb/ot chain
                        nc.scalar.activation(
                            ob[:, n * 512:(n + 1) * 512], pc[:],
                            mybir.ActivationFunctionType.Copy)
                    else:
                        nc.vector.tensor_copy(
                            ob[:, n * 512:(n + 1) * 512], pc[:])
                    m = 4 * jj + mo
                    if tail:
                        # one full-tile store: fewer HWDGE issue slots at
                        # the end of the kernel
                        if n == 1:
                            nc.sync.dma_start(out_d[m * P:(m + 1) * P, :],
                                              ob[:])
                    else:
                        nc.sync.dma_start(
                            out_d[m * P:(m + 1) * P, n * 512:(n + 1) * 512],
                            ob[:, n * 512:(n + 1) * 512])
                return (it, 430, 0)

            obs = {}

            # ---------- schedule ----------
            # master stream: all S tiles in (q, hs, i) order; ACT must never
            # starve, PE (the bottleneck) must never park behind a stalled
            # S matmul. Fillers are paced by a global PE-vs-ACT cost ledger;
            # per-phase filler assignment matches each segment's PE deficit
            # (~611ns per S tile).
            def weave(masters, fillers, extra=()):
                act_t, pe_t, fi = 0.0, 0.0, 0
                for k, (it, pe, act) in enumerate(masters):
                    it()
                    act_t += act
                    pe_t += pe
                    while fi < len(fillers) and pe_t + 400 < act_t:
                        f, fpe, _ = fillers[fi]
                        f()
                        pe_t += fpe
                        fi += 1
                    for pos, item in extra:
                        if pos == k:
                            item[0]()
                            pe_t += item[1]
                while fi < len(fillers):
                    f, fpe, _ = fillers[fi]
                    f()
                    fi += 1

            def pv_pair(q, hs, qc):
                return [pv_item(q, hs, qc, 0), pv_item(q, hs, qc, 1)]

            # prologue: q=0 projections at term granularity so PE starts as
            # soon as the first weight chunk lands
            def qk_term(q, wpfx, s_, ti, dsts=None, bias=None):
                nms = (wpfx + "h", wpfx + "1", wpfx + "l")
                xsrcs = (xhs_t, xls_t, xhs_t)
                key = ("pro", q, wpfx, s_)

                def it():
                    if ti == 0:
                        pro_ps[key] = psU.tile([P, 512], f32, tag="ps1",
                                               name=f"pq{q}_{wpfx}_{s_}")
                    pq = pro_ps[key]
                    w8 = wt[nms[ti]]
                    for t in range(KO // 2):
                        nc.tensor.matmul(
                            pq[:],
                            w8[:, s_, 2 * t:2 * t + 2, :],
                            x_mv(xsrcs[ti], q, t),
                            start=(ti == 0 and t == 0),
                            stop=(ti == 2 and t == KO // 2 - 1),
                            perf_mode=DR)
                    if ti == 2:
                        nc.vector.tensor_scalar_add(
                            dsts[s_][q][:], pq[:], bias[:, s_:s_ + 1])
                return (it, 430, 0)

            pro_ps = {}
            for ti in range(3):
                qk_term(0, "wq", 0, ti, qts, bqs)[0]()
                qk_term(0, "wk", 0, ti, kts, bks)[0]()

            masters0 = [s_item(0, 0, i) for i in range(4)]
            fillers0 = [qk8_item(0, "wq", qts, bqs, 1),
                        qk8_item(0, "wk", kts, bks, 1)]
            weave(masters0, fillers0)

            # per-phase filler assignment (see header comment)
            FA = {1: [], 2: [], 3: []}
            FB = {0: [], 1: [], 2: [], 3: []}
            # phase 0 B: v8(0), PV(0,0), tr, A(1), v8(1)
            FB[0] += [v8_item(0, 0), v8_item(0, 1)]
            for qc in range(4):
                FB[0] += pv_pair(0, 0, qc)
                if qc == 0:
                    FB[0] += [v8_item(0, 2), v8_item(0, 3)]
                if qc >= 1:
                    FB[0].append(tr_item(0, 0, qc - 1))
            FB[0].append(tr_item(0, 0, 3))
            FB[0] += qk8_items(1)
            FB[0] += [v8_item(1, k) for k in range(4)]
            # phase 1 A: PV(0,1), tr, A(2) s0-half, v8(2)
            for qc in range(4):
                FA[1] += pv_pair(0, 1, qc)
                if qc >= 1:
                    FA[1].append(tr_item(0, 1, qc - 1))
            FA[1].append(tr_item(0, 1, 3))
            FA[1] += qk8_items(2)[0:2]
            FA[1] += [v8_item(2, k) for k in range(4)]
            # phase 1 B: PV(1,0), tr, A(2) s1-half
            for qc in range(4):
                FB[1] += pv_pair(1, 0, qc)
                if qc >= 1:
                    FB[1].append(tr_item(1, 0, qc - 1))
            FB[1].append(tr_item(1, 0, 3))
            FB[1] += qk8_items(2)[2:4]
            # phase 2 A: PV(1,1), tr, A(3) s0-half, v8(3)
            for qc in range(4):
                FA[2] += pv_pair(1, 1, qc)
                if qc >= 1:
                    FA[2].append(tr_item(1, 1, qc - 1))
            FA[2].append(tr_item(1, 1, 3))
            FA[2] += qk8_items(3)[0:2]
            FA[2] += [v8_item(3, k) for k in range(4)]
            # phase 2 B: PV(2,0), tr, A(3) s1-half
            for qc in range(4):
                FB[2] += pv_pair(2, 0, qc)
                if qc >= 1:
                    FB[2].append(tr_item(2, 0, qc - 1))
            FB[2].append(tr_item(2, 0, 3))
            FB[2] += qk8_items(3)[2:4]
            # phase 3 A: PV(2,1), tr, c(1) all, c(2) all
            for qc in range(4):
                FA[3] += pv_pair(2, 1, qc)
                if qc >= 1:
                    FA[3].append(tr_item(2, 1, qc - 1))
            FA[3].append(tr_item(2, 1, 3))
            for mo in range(4):
                FA[3] += [c_item(1, mo, 0), c_item(1, mo, 1)]
            for mo in range(4):
                FA[3] += [c_item(2, mo, 0), c_item(2, mo, 1)]
            # phase 3 B: PV(3,0), tr, c(0) all; PV(3,1,0..2) placed late
            # via `extra` (they need the last exps of this segment)
            for qc in range(4):
                FB[3] += pv_pair(3, 0, qc)
                if qc >= 1:
                    FB[3].append(tr_item(3, 0, qc - 1))
            FB[3].append(tr_item(3, 0, 3))
            for mo in range(4):
                FB[3] += [c_item(0, mo, 0), c_item(0, mo, 1)]

            for q in range(4):
                if q > 0:
                    mastersA = [s_item(q, 0, i) for i in range(4 * q + 4)]
                    weave(mastersA, FA[q])
                mastersB = [s_item(q, 1, i) for i in range(4 * q + 4)]
                if q == 3:
                    extra = [(14, pv_item(3, 1, 0, 0)),
                             (14, pv_item(3, 1, 0, 1)),
                             (15, pv_item(3, 1, 1, 0)),
                             (15, pv_item(3, 1, 1, 1)),
                             (15, tr_item(3, 1, 0))]
                    weave(mastersB, FB[q], extra)
                else:
                    weave(mastersB, FB[q])

            # tail: remaining PV(3,1), transposes, c(3,*) staggered
            pv_item(3, 1, 2, 0)[0]()
            pv_item(3, 1, 2, 1)[0]()
            tr_item(3, 1, 1)[0]()
            c_item(3, 0, 0, tail=True)[0]()
            c_item(3, 0, 1, tail=True)[0]()
            pv_item(3, 1, 3, 0)[0]()
            pv_item(3, 1, 3, 1)[0]()
            tr_item(3, 1, 2)[0]()
            c_item(3, 1, 0, tail=True)[0]()
            c_item(3, 1, 1, tail=True)[0]()
            tr_item(3, 1, 3)[0]()
            c_item(3, 2, 0, tail=True)[0]()
            c_item(3, 2, 1, tail=True)[0]()
            c_item(3, 3, 0, tail=True)[0]()
            c_item(3, 3, 1, tail=True)[0]()

    nc.compile()
    return nc


def _get_nc():
    if "nc" not in _CACHE:
        _CACHE["nc"] = _build()
    return _CACHE["nc"]


def _get_runner():
    """Build the jitted SPMD executor once (mirrors bass2jax.run_bass_via_pjrt
    but caches the jitted function so repeat calls skip retrace/recompile)."""
    if "runner" in _CACHE:
        return _CACHE["runner"]
    import jax
    import numpy as _np
    from jax.sharding import Mesh, PartitionSpec
    from jax.experimental.shard_map import shard_map
    import concourse.mybir as mybir
    from concourse import bass2jax

    nc = _get_nc()
    bass2jax.install_neuronx_cc_hook()

    partition_name = (nc.partition_id_tensor.name
                      if nc.partition_id_tensor else None)
    in_names, out_names, out_avals, zero_shapes = [], [], [], []
    for alloc in nc.m.functions[0].allocations:
        if not isinstance(alloc, mybir.MemoryLocationSet):
            continue
        name = alloc.memorylocations[0].name
        if alloc.kind == "ExternalInput":
            if name != partition_name:
                in_names.append(name)
        elif alloc.kind == "ExternalOutput":
            out_avals.append(jax.core.ShapedArray(
                tuple(alloc.tensor_shape), mybir.dt.np(alloc.dtype)))
            out_names.append(name)
            zero_shapes.append((tuple(alloc.tensor_shape),
                                mybir.dt.np(alloc.dtype)))
    n_params = len(in_names)
    n_outs = len(out_names)
    all_names = in_names + out_names
    if partition_name is not None:
        all_names = all_names + [partition_name]

    def _body(*args):
        operands = list(args)
        if partition_name is not None:
            operands.append(bass2jax.partition_id_tensor())
        outs = bass2jax._bass_exec_p.bind(
            *operands,
            out_avals=tuple(out_avals),
            in_names=tuple(all_names),
            out_names=tuple(out_names),
            lowering_input_output_aliases=(),
            sim_require_finite=True,
            sim_require_nnan=True,
            nc=nc,
        )
        return tuple(outs)

    devices = jax.devices()[:N_CORES]
    mesh = Mesh(_np.asarray(devices), ("core",))
    donate = tuple(range(n_params, n_params + n_outs))
    sharded = jax.jit(
        shard_map(_body, mesh=mesh,
                  in_specs=(PartitionSpec("core"),) * (n_params + n_outs),
                  out_specs=(PartitionSpec("core"),) * n_outs,
                  check_rep=False),
        donate_argnums=donate, keep_unused=True)

    def run(in_maps):
        concat_in = [
            _np.concatenate([_np.asarray(m[name]) for m in in_maps], axis=0)
            for name in in_names]
        concat_zeros = [
            _np.zeros((N_CORES * sh[0], *sh[1:]), dtype)
            for sh, dtype in zero_shapes]
        out_arrs = sharded(*concat_in, *concat_zeros)
        return [
            {name: _np.asarray(out_arrs[i]).reshape(
                N_CORES, *zero_shapes[i][0])[c]
             for i, name in enumerate(out_names)}
            for c in range(N_CORES)]

    _CACHE["runner"] = run
    return run


def _fp8():
    return (ml_dtypes.float8_e4m3fn if hasattr(ml_dtypes, 'float8_e4m3fn')
            else ml_dtypes.float8_e4m3)


def kernel(x, mask, W_qkv, b_qkv, W_out, b_out):
    bf = ml_dtypes.bfloat16
    f8 = _fp8()
    x = np.asarray(x, dtype=np.float32)
    W_qkv = np.asarray(W_qkv, dtype=np.float32)
    b_qkv = np.asarray(b_qkv, dtype=np.float32)
    W_out = np.asarray(W_out, dtype=np.float32)
    b_out = np.asarray(b_out, dtype=np.float32)
    # mask is the causal tril mask (per problem spec); causality is
    # implemented structurally on-device.

    run = _get_runner()

    def pack(wslice):
        # [C, DH] -> [P, 2, KO, P] with C = ko*P + p, DH = s*P + d
        return np.ascontiguousarray(
            wslice.reshape(KO, P, 2, P).transpose(1, 2, 0, 3))

    def w_hilo(wslice):
        # hi/lo fp8 split: 32W ~= Wh + (Wl term via xh) with W1 for xl term
        w32 = wslice * WSCALE
        wh = w32.astype(f8)
        wl = (w32 - wh.astype(np.float32)).astype(f8)
        w1 = wslice.astype(f8)
        return (pack(wh.astype(np.float32)).astype(f8),
                pack(w1.astype(np.float32)).astype(f8),
                pack(wl.astype(np.float32)).astype(f8))

    def pack_b(bslice):
        # [DH] -> [P, 2] with idx = s*P + p, x32
        return np.ascontiguousarray(
            (bslice * WSCALE).reshape(2, P).T).astype(np.float32)

    tri = np.triu(np.ones((P, P), dtype=np.float32))
    tri2 = np.ascontiguousarray(
        np.broadcast_to(tri[:, None, :], (P, 2, P))).astype(bf)
    ident = np.eye(P, dtype=np.float32).astype(bf)

    xhs, xls = [], []
    for b in range(B):
        xt = np.ascontiguousarray(x[b].T)  # [C, T]
        xh = xt.astype(f8)
        xl = ((xt - xh.astype(np.float32)) * WSCALE).astype(f8)
        xhs.append(xh)
        xls.append(xl)

    in_maps = []
    for core in range(N_CORES):
        b, g = divmod(core, GROUPS)
        cs = slice(g * DH, (g + 1) * DH)
        wq3 = w_hilo(W_qkv[:, cs])
        wk3 = w_hilo(W_qkv[:, C:][:, cs])
        wv3 = w_hilo(W_qkv[:, 2 * C:][:, cs])
        in_maps.append({
            "xh": xhs[b], "xl": xls[b],
            "wqh": wq3[0], "wq1": wq3[1], "wql": wq3[2],
            "wkh": wk3[0], "wk1": wk3[1], "wkl": wk3[2],
            "wvh": wv3[0], "wv1": wv3[1], "wvl": wv3[2],
            "wo": np.ascontiguousarray(
                W_out[cs, :].reshape(2, P, C).transpose(1, 0, 2)).astype(bf),
            "bq": pack_b(b_qkv[cs]),
            "bk": pack_b(b_qkv[C:][cs]),
            "bv": np.ascontiguousarray(
                (b_qkv[2 * C:][cs] * WSCALE)[None, :]).astype(np.float32),
            "tri": tri2,
            "ident": ident,
        })

    results = run(in_maps)

    out = np.zeros((B, T, C), dtype=np.float32)
    for core in range(N_CORES):
        b = core // GROUPS
        out[b] += results[core]["out"].astype(np.float32)
    out += b_out[None, None, :]
    return out


# revision 49
# speedup vs baseline: 1.0381x; 1.0381x over previous
"""Trainium2 Bass kernel for nn_CausalSelfAttention (B=2, T=2048, C=1024, 16 heads).

Sharding: 8 cores = 2 batches x 4 head-groups (4 heads each).

v3 design (cost-model driven):
  - QKV projections run as fp8(e4m3) DoubleRow matmuls with hi/lo error
    compensation: 32*x@W = xh@fp8(32W) + fp8(32(x-xh))@fp8(W)
    + xh@fp8(32W - fp8(32W)), all three terms accumulated in one PSUM
    group. DR charges 0.5 cycles/output-col and packs 2 contraction
    chunks per instruction, so this costs 0.75x of bf16 at ~0.3% error
    (plain fp8 would be 0.25x cost but ~5% error -- softmax averaging
    shrinks signal and noise equally, so that error hits the output
    full-strength and blows the 2e-2 gate).
  - S = Q^T K, P (exp output), P@V, and the out-projection stay bf16.
  - exp runs on ACT (~73us busy); PE (~90us busy) is the bottleneck, the
    emission weaves S tiles and filler work by a cost ledger.
  - diagonal causal masking: post-exp multiply by triu-ones on the
    (otherwise idle) gpsimd/Pool engine.
  - O is normalized in [q,d] layout then transposed via PE (identity
    matmul) + DVE copy -- no DMA XBAR on the critical path.
  - out-projection in bf16, partial [T,C] per core; host sums the 4
    tensor-parallel partials per batch and adds b_out.
"""
import sys

if '/opt/trn_rl_repo' not in sys.path:
    sys.path.insert(0, '/opt/trn_rl_repo')

import numpy as np
import ml_dtypes

B, T, C = 2, 2048, 1024
N_HEAD = 16
D = 64
P = 128
N_CORES = 8
GROUPS = N_CORES // B            # 4 tensor-parallel groups per batch
HPC = N_HEAD // GROUPS           # 4 heads per core
DH = HPC * D                     # 256 head dims per core
KO = C // P                      # 8 contraction subtiles for projections
NQB = T // 512                   # 4 q blocks of 512
WSCALE = 32.0                    # fp8 range scaling of W_qkv
SCALE = 1.0 / (np.sqrt(D) * WSCALE * WSCALE)   # exp scale

_CACHE = {}

# weave pacing costs (ns)
PE_CYC = 0.4167


def _build():
    import concourse.mybir as mybir
    import concourse.tile as tile
    from concourse import bacc

    f32 = mybir.dt.float32
    bf16 = mybir.dt.bfloat16
    f16 = mybir.dt.float16
    fp8 = mybir.dt.float8e4
    DR = mybir.MatmulPerfMode.DoubleRow
    EXP = mybir.ActivationFunctionType.Exp
    MUL = mybir.AluOpType.mult
    ADD = mybir.AluOpType.add

    nc = bacc.Bacc("TRN2", target_bir_lowering=False, debug=False,
                   num_devices=N_CORES)

    xh_d = nc.dram_tensor("xh", [C, T], fp8, kind="ExternalInput")
    xl_d = nc.dram_tensor("xl", [C, T], fp8, kind="ExternalInput")
    w_ds = {}
    for nm in ("wqh", "wq1", "wql", "wkh", "wk1", "wkl",
               "wvh", "wv1", "wvl"):
        w_ds[nm] = nc.dram_tensor(nm, [P, 2, KO, P], fp8,
                                  kind="ExternalInput")
    wo_d = nc.dram_tensor("wo", [P, 2, C], bf16, kind="ExternalInput")
    bq_d = nc.dram_tensor("bq", [P, 2], f32, kind="ExternalInput")
    bk_d = nc.dram_tensor("bk", [P, 2], f32, kind="ExternalInput")
    bv_d = nc.dram_tensor("bv", [1, DH], f32, kind="ExternalInput")
    tri_d = nc.dram_tensor("tri", [P, 2, P], bf16, kind="ExternalInput")
    id_d = nc.dram_tensor("ident", [P, P], bf16, kind="ExternalInput")
    out_d = nc.dram_tensor("out", [T, C], f16, kind="ExternalOutput")

    pls = [slice(0, D), slice(D, 2 * D)]

    with tile.TileContext(nc) as tc:
        with (
            tc.tile_pool(name="pp", bufs=1) as pp,
            tc.tile_pool(name="wS", bufs=4) as wS,
            tc.tile_pool(name="wO", bufs=4) as wO,
            tc.tile_pool(name="psS", bufs=2, space="PSUM") as psS,
            tc.tile_pool(name="psU", bufs=4, space="PSUM") as psU,
        ):
            xhs_t = [pp.tile([P, KO, 512], fp8, tag=f"xh{q}", name=f"xh{q}")
                     for q in range(4)]
            xls_t = [pp.tile([P, KO, 512], fp8, tag=f"xl{q}", name=f"xl{q}")
                     for q in range(4)]

            def x_mv(xt_, q, t):
                # moving operand [p, 2(ko-pair), 512]
                return xt_[q][:, 2 * t:2 * t + 2, :]

            def x_st(xt_, q, t, it_):
                # stationary operand [p, 2(ko-pair), 128]
                return xt_[q][:, 2 * t:2 * t + 2, it_ * P:(it_ + 1) * P]
            wt = {nm: pp.tile([P, 2, KO, P], fp8, tag=nm, name=nm)
                  for nm in ("wqh", "wq1", "wql", "wkh", "wk1", "wkl",
                             "wvh", "wv1", "wvl")}
            wos = pp.tile([P, 2, C], bf16, tag="wos")
            qts = [[pp.tile([P, 512], bf16, tag=f"qt{s}_{q}", name=f"qt{s}_{q}")
                    for q in range(4)] for s in range(2)]
            kts = [[pp.tile([P, 512], bf16, tag=f"kt{s}_{q}", name=f"kt{s}_{q}")
                    for q in range(4)] for s in range(2)]
            # vo: [k-part, kt-slot, head(2hs+u), d + ones-col]
            vo = pp.tile([P, 16, HPC, D + 1], bf16, tag="vo")
            # pt: per hs P^T tiles [k-part, kt-slot, u, q-col of current block]
            pts = [pp.tile([P, 16, 2, 512], bf16, tag=f"pt{hs}", name=f"pt{hs}")
                   for hs in range(2)]
            ots = [[pp.tile([P, 512], bf16, tag=f"ot{j}_{hs}",
                            name=f"ot{j}_{hs}") for hs in range(2)]
                   for j in range(NQB)]
            bqs = pp.tile([P, 2], f32, tag="bqs")
            bks = pp.tile([P, 2], f32, tag="bks")
            bvrow = pp.tile([1, DH], f32, tag="bvrow")
            bvb = pp.tile([P, DH], f32, tag="bvb")
            trib = pp.tile([P, 2, P], bf16, tag="trib")
            ident = pp.tile([P, P], bf16, tag="ident")
            scr = pp.tile([1, 1], f32, tag="scr")

            # trigger the exp ACT-table load immediately (scratch memset)
            nc.vector.memset(scr[:], 0.0)
            nc.scalar.activation(scr[0:1, 0:1], scr[0:1, 0:1], EXP)
            # ones-column (=WSCALE) of V for softmax denominators
            nc.vector.memset(vo[:, :, :, D:D + 1], WSCALE)

            # ---- loads (sync=SP queue, scalar=ACT queue, gpsimd=SWDGE) ----
            xh_r = xh_d.rearrange("(ko p) t -> p ko t", p=P)
            xl_r = xl_d.rearrange("(ko p) t -> p ko t", p=P)
            # critical chain (first S tile): wqh,xh0,xl0,wq1,wql + wk* on
            # SWDGE. Big non-critical loads go late; sync gets tiny loads
            # so the round-robin HWDGE issue doesn't let x quarters jump
            # ahead of critical weight loads.
            for nm in ("wqh", "wq1", "wql"):
                nc.sync.dma_start(wt[nm][:, 0], w_ds[nm][:, 0])
            nc.scalar.dma_start(xhs_t[0][:], xh_r[:, :, 0:512])
            nc.scalar.dma_start(xls_t[0][:], xl_r[:, :, 0:512])
            for nm in ("wkh", "wk1", "wkl"):
                nc.gpsimd.dma_start(wt[nm][:, 0], w_ds[nm][:, 0])
            nc.sync.dma_start(bqs[:], bq_d[:])
            nc.sync.dma_start(bks[:], bk_d[:])
            for nm in ("wqh", "wq1", "wql"):
                nc.sync.dma_start(wt[nm][:, 1], w_ds[nm][:, 1])
            for nm in ("wkh", "wk1", "wkl"):
                nc.gpsimd.dma_start(wt[nm][:, 1], w_ds[nm][:, 1])
            for nm in ("wvh", "wv1", "wvl"):
                nc.gpsimd.dma_start(wt[nm][:], w_ds[nm][:])
            nc.sync.dma_start(bvrow[:], bv_d[:])
            nc.sync.dma_start(trib[:], tri_d[:])
            nc.sync.dma_start(ident[:], id_d[:])
            for q in range(1, 4):
                nc.scalar.dma_start(xhs_t[q][:],
                                    xh_r[:, :, q * 512:(q + 1) * 512])
                nc.scalar.dma_start(xls_t[q][:],
                                    xl_r[:, :, q * 512:(q + 1) * 512])
            nc.sync.dma_start(wos[:], wo_d[:])

            nc.gpsimd.partition_broadcast(bvb[:, :], bvrow[0:1, :])

            # ---------- item constructors (thunk, pe_ns, act_ns) ----------
            # hi/lo fp8 compensation: 32xW = xh@Wh + xl@W1 + xh@Wl, all
            # accumulated in one PSUM group (12 DR steps).
            def qk8_item(q, wpfx, dsts, bias, s_):
                terms = [(wt[wpfx + "h"], xhs_t), (wt[wpfx + "1"], xls_t),
                         (wt[wpfx + "l"], xhs_t)]

                def it():
                    pq = psU.tile([P, 512], f32, tag="ps1",
                                  name=f"pq{q}_{wpfx}_{s_}")
                    for ti, (w8, xsrc) in enumerate(terms):
                        for t in range(KO // 2):
                            nc.tensor.matmul(
                                pq[:],
                                w8[:, s_, 2 * t:2 * t + 2, :],
                                x_mv(xsrc, q, t),
                                start=(ti == 0 and t == 0),
                                stop=(ti == 2 and t == KO // 2 - 1),
                                perf_mode=DR)
                    nc.vector.tensor_scalar_add(
                        dsts[s_][q][:], pq[:], bias[:, s_:s_ + 1])
                return (it, 1290, 0)

            def qk8_items(q):
                # order: Q s0, K s0, Q s1, K s1 (heads-split 0 first so the
                # next phase's S(q,0) stream unblocks earliest)
                return [qk8_item(q, "wq", qts, bqs, 0),
                        qk8_item(q, "wk", kts, bks, 0),
                        qk8_item(q, "wq", qts, bqs, 1),
                        qk8_item(q, "wk", kts, bks, 1)]

            def v8_item(q, it_):
                terms = [(wt["wvh"], xhs_t), (wt["wv1"], xls_t),
                         (wt["wvl"], xhs_t)]

                def it():
                    pv = psU.tile([P, DH], f32, tag="ps1",
                                  name=f"pv{q}_{it_}")
                    for ti, (w8, xsrc) in enumerate(terms):
                        for t in range(KO // 2):
                            nc.tensor.matmul(
                                pv[:],
                                x_st(xsrc, q, t, it_),
                                w8[:, :, 2 * t:2 * t + 2, :].rearrange(
                                    "p s k d -> p k s d"),
                                start=(ti == 0 and t == 0),
                                stop=(ti == 2 and t == KO // 2 - 1),
                                perf_mode=DR)
                    nc.vector.tensor_tensor(
                        vo[:, 4 * q + it_, :, 0:D],
                        pv[:].rearrange("p (h d) -> p h d", h=HPC),
                        bvb.rearrange("p (h d) -> p h d", h=HPC),
                        ADD)
                return (it, 645, 0)

            def s_item(q, hs, i):
                off = max(0, P * i - 512 * q)
                diag = P * i >= 512 * q

                def it():
                    sp = psS.tile([P, 2, 512], f32, tag="sp",
                                  name=f"sp{q}_{hs}_{i}")
                    for u in range(2):
                        nc.tensor.matmul(
                            sp[:, u, off:512],
                            kts[hs][i // 4][pls[u],
                                            (i % 4) * P:(i % 4 + 1) * P],
                            qts[hs][q][pls[u], off:512],
                            start=True, stop=True)
                    nc.scalar.activation(pts[hs][:, i, :, off:512],
                                         sp[:, :, off:512],
                                         EXP, scale=float(SCALE))
                    if diag:
                        nc.gpsimd.tensor_tensor(
                            pts[hs][:, i, :, off:off + P],
                            pts[hs][:, i, :, off:off + P],
                            trib[:], MUL)
                w = 512 - off
                return (it, 2 * w * PE_CYC + 20, 2 * w * 0.833 + 190)

            def pv_item(q, hs, qc, u):
                qt = 4 * q + qc

                def it():
                    po = psU.tile([P, 512], f32, tag="ps1",
                                  name=f"po{q}_{hs}_{qc}_{u}")
                    for kt in range(qt + 1):
                        nc.tensor.matmul(
                            po[:, 0:D + 1],
                            pts[hs][:, kt, u, qc * P:(qc + 1) * P],
                            vo[:, kt, 2 * hs + u, :],
                            start=(kt == 0), stop=(kt == qt))
                    rd = wS.tile([P, 1], f32, tag="rd",
                                 name=f"rd{q}_{hs}_{qc}_{u}")
                    if u == 0:
                        osbs[(q, hs, qc)] = wS.tile(
                            [P, 2, D], bf16, tag="osb",
                            name=f"osb{q}_{hs}_{qc}")
                    osb = osbs[(q, hs, qc)]
                    nc.vector.reciprocal_approx_fast(rd[:], po[:, D:D + 1])
                    nc.vector.tensor_scalar_mul(
                        osb[:, u, :], po[:, 0:D], rd[:])
                return (it, (qt + 1) * 27 + 30, 0)

            osbs = {}

            def tr_item(q, hs, qc):
                def it():
                    ptr = psU.tile([P, P], bf16, tag="ps1",
                                   name=f"ptr{q}_{hs}_{qc}")
                    nc.tensor.transpose(
                        ptr[:], osbs[(q, hs, qc)].rearrange("p u d -> p (u d)"),
                        ident[:])
                    nc.vector.tensor_copy(
                        ots[q][hs][:, qc * P:(qc + 1) * P], ptr[:])
                return (it, 75, 0)

            def c_item(jj, mo, n, tail=False):
                def it():
                    pc = psU.tile([P, 512], f32, tag="ps1",
                                  name=f"pc{jj}_{mo}_{n}")
                    for s in range(2):
                        nc.tensor.matmul(
                            pc[:],
                            ots[jj][s][:, mo * P:(mo + 1) * P],
                            wos[:, s, n * 512:(n + 1) * 512],
                            start=(s == 0), stop=(s == 1))
                    if n == 0:
                        obs[(jj, mo)] = wO.tile([P, C], f16, tag="ob",
                                                name=f"ob{jj}_{mo}")
                    ob = obs[(jj, mo)]
                    if tail:
                        # ACT is idle after the last exp; offload the copy
                        # so DVE can serve the critical osb/ot chain
                        nc.scalar.activation(
                            ob[:, n * 512:(n + 1) * 512], pc[:],
                            mybir.ActivationFunctionType.Copy)
                    else:
                        nc.vector.tensor_copy(
                            ob[:, n * 512:(n + 1) * 512], pc[:])
                    m = 4 * jj + mo
                    if tail:
                        # one full-tile store: fewer HWDGE issue slots at
                        # the end of the kernel
                        if n == 1:
                            nc.sync.dma_start(out_d[m * P:(m + 1) * P, :],
                                              ob[:])
                    else:
                        nc.sync.dma_start(
                            out_d[m * P:(m + 1) * P, n * 512:(n + 1) * 512],
                            ob[:, n * 512:(n + 1) * 512])
                return (it, 430, 0)

            obs = {}

            # ---------- schedule ----------
            # master stream: all S tiles in (q, hs, i) order; ACT must never
            # starve, PE (the bottleneck) must never park behind a stalled
            # S matmul. Fillers are paced by a global PE-vs-ACT cost ledger;
            # per-phase filler assignment matches each segment's PE deficit
            # (~611ns per S tile).
            def weave(masters, fillers, extra=()):
                act_t, pe_t, fi = 0.0, 0.0, 0
                for k, (it, pe, act) in enumerate(masters):
                    it()
                    act_t += act
                    pe_t += pe
                    while fi < len(fillers) and pe_t + 400 < act_t:
                        f, fpe, _ = fillers[fi]
                        f()
                        pe_t += fpe
                        fi += 1
                    for pos, item in extra:
                        if pos == k:
                            item[0]()
                            pe_t += item[1]
                while fi < len(fillers):
                    f, fpe, _ = fillers[fi]
                    f()
                    fi += 1

            def pv_pair(q, hs, qc):
                return [pv_item(q, hs, qc, 0), pv_item(q, hs, qc, 1)]

            # prologue: q=0 projections at term granularity so PE starts as
            # soon as the first weight chunk lands
            def qk_term(q, wpfx, s_, ti, dsts=None, bias=None):
                nms = (wpfx + "h", wpfx + "1", wpfx + "l")
                xsrcs = (xhs_t, xls_t, xhs_t)
                key = ("pro", q, wpfx, s_)

                def it():
                    if ti == 0:
                        pro_ps[key] = psU.tile([P, 512], f32, tag="ps1",
                                               name=f"pq{q}_{wpfx}_{s_}")
                    pq = pro_ps[key]
                    w8 = wt[nms[ti]]
                    for t in range(KO // 2):
                        nc.tensor.matmul(
                            pq[:],
                            w8[:, s_, 2 * t:2 * t + 2, :],
                            x_mv(xsrcs[ti], q, t),
                            start=(ti == 0 and t == 0),
                            stop=(ti == 2 and t == KO // 2 - 1),
                            perf_mode=DR)
                    if ti == 2:
                        nc.vector.tensor_scalar_add(
                            dsts[s_][q][:], pq[:], bias[:, s_:s_ + 1])
                return (it, 430, 0)

            pro_ps = {}
            for ti in range(3):
                qk_term(0, "wq", 0, ti, qts, bqs)[0]()
                qk_term(0, "wk", 0, ti, kts, bks)[0]()

            masters0 = [s_item(0, 0, i) for i in range(4)]
            fillers0 = [qk8_item(0, "wq", qts, bqs, 1),
                        qk8_item(0, "wk", kts, bks, 1)]
            weave(masters0, fillers0)

            # per-phase filler assignment (see header comment)
            FA = {1: [], 2: [], 3: []}
            FB = {0: [], 1: [], 2: [], 3: []}
            # phase 0 B: v8(0), PV(0,0), tr, A(1), v8(1)
            FB[0] += [v8_item(0, 0), v8_item(0, 1)]
            for qc in range(4):
                FB[0] += pv_pair(0, 0, qc)
                if qc == 0:
                    FB[0] += [v8_item(0, 2), v8_item(0, 3)]
                if qc >= 1:
                    FB[0].append(tr_item(0, 0, qc - 1))
            FB[0].append(tr_item(0, 0, 3))
            FB[0] += qk8_items(1)
            FB[0] += [v8_item(1, k) for k in range(4)]
            def pv_block(q_, hs_):
                blk = []
                for qc in range(4):
                    blk += pv_pair(q_, hs_, qc)
                    if qc >= 1:
                        blk.append(tr_item(q_, hs_, qc - 1))
                blk.append(tr_item(q_, hs_, 3))
                return blk

            def seg(pvb, others):
                # 2 non-PV fillers first (PV waits on the freshest exps)
                return others[:2] + pvb + others[2:]

            # phase 1 A: PV(0,1)+tr, A(2) s0-half, v8(2)
            FA[1] = seg(pv_block(0, 1),
                        qk8_items(2)[0:2] + [v8_item(2, k) for k in range(4)])
            # phase 1 B: PV(1,0)+tr, A(2) s1-half
            FB[1] = seg(pv_block(1, 0), qk8_items(2)[2:4])
            # phase 2 A: PV(1,1)+tr, A(3) s0-half, v8(3)
            FA[2] = seg(pv_block(1, 1),
                        qk8_items(3)[0:2] + [v8_item(3, k) for k in range(4)])
            # phase 2 B: PV(2,0)+tr, A(3) s1-half
            FB[2] = seg(pv_block(2, 0), qk8_items(3)[2:4])
            # phase 3 A: PV(2,1)+tr, c(1) all, c(2) all
            FA[3] = seg(pv_block(2, 1),
                        [c_item(1, mo, n) for mo in range(4) for n in (0, 1)]
                        + [c_item(2, mo, n) for mo in range(4)
                           for n in (0, 1)])
            # phase 3 B: PV(3,0)+tr, c(0) all; PV(3,1,0..2) placed late
            # via `extra` (they need the last exps of this segment)
            FB[3] = seg(pv_block(3, 0),
                        [c_item(0, mo, n) for mo in range(4) for n in (0, 1)])

            for q in range(4):
                if q > 0:
                    mastersA = [s_item(q, 0, i) for i in range(4 * q + 4)]
                    weave(mastersA, FA[q])
                mastersB = [s_item(q, 1, i) for i in range(4 * q + 4)]
                if q == 3:
                    extra = [(14, pv_item(3, 1, 0, 0)),
                             (14, pv_item(3, 1, 0, 1)),
                             (15, pv_item(3, 1, 1, 0)),
                             (15, pv_item(3, 1, 1, 1)),
                             (15, tr_item(3, 1, 0))]
                    weave(mastersB, FB[q], extra)
                else:
                    weave(mastersB, FB[q])

            # tail: remaining PV(3,1), transposes, c(3,*) staggered
            pv_item(3, 1, 2, 0)[0]()
            pv_item(3, 1, 2, 1)[0]()
            tr_item(3, 1, 1)[0]()
            c_item(3, 0, 0, tail=True)[0]()
            c_item(3, 0, 1, tail=True)[0]()
            pv_item(3, 1, 3, 0)[0]()
            pv_item(3, 1, 3, 1)[0]()
            tr_item(3, 1, 2)[0]()
            c_item(3, 1, 0, tail=True)[0]()
            c_item(3, 1, 1, tail=True)[0]()
            tr_item(3, 1, 3)[0]()
            c_item(3, 2, 0, tail=True)[0]()
            c_item(3, 2, 1, tail=True)[0]()
            c_item(3, 3, 0, tail=True)[0]()
            c_item(3, 3, 1, tail=True)[0]()

    nc.compile()
    return nc


def _get_nc():
    if "nc" not in _CACHE:
        _CACHE["nc"] = _build()
    return _CACHE["nc"]


def _get_runner():
    """Build the jitted SPMD executor once (mirrors bass2jax.run_bass_via_pjrt
    but caches the jitted function so repeat calls skip retrace/recompile)."""
    if "runner" in _CACHE:
        return _CACHE["runner"]
    import jax
    import numpy as _np
    from jax.sharding import Mesh, PartitionSpec
    from jax.experimental.shard_map import shard_map
    import concourse.mybir as mybir
    from concourse import bass2jax

    nc = _get_nc()
    bass2jax.install_neuronx_cc_hook()

    partition_name = (nc.partition_id_tensor.name
                      if nc.partition_id_tensor else None)
    in_names, out_names, out_avals, zero_shapes = [], [], [], []
    for alloc in nc.m.functions[0].allocations:
        if not isinstance(alloc, mybir.MemoryLocationSet):
            continue
        name = alloc.memorylocations[0].name
        if alloc.kind == "ExternalInput":
            if name != partition_name:
                in_names.append(name)
        elif alloc.kind == "ExternalOutput":
            out_avals.append(jax.core.ShapedArray(
                tuple(alloc.tensor_shape), mybir.dt.np(alloc.dtype)))
            out_names.append(name)
            zero_shapes.append((tuple(alloc.tensor_shape),
                                mybir.dt.np(alloc.dtype)))
    n_params = len(in_names)
    n_outs = len(out_names)
    all_names = in_names + out_names
    if partition_name is not None:
        all_names = all_names + [partition_name]

    def _body(*args):
        operands = list(args)
        if partition_name is not None:
            operands.append(bass2jax.partition_id_tensor())
        outs = bass2jax._bass_exec_p.bind(
            *operands,
            out_avals=tuple(out_avals),
            in_names=tuple(all_names),
            out_names=tuple(out_names),
            lowering_input_output_aliases=(),
            sim_require_finite=True,
            sim_require_nnan=True,
            nc=nc,
        )
        return tuple(outs)

    devices = jax.devices()[:N_CORES]
    mesh = Mesh(_np.asarray(devices), ("core",))
    donate = tuple(range(n_params, n_params + n_outs))
    sharded = jax.jit(
        shard_map(_body, mesh=mesh,
                  in_specs=(PartitionSpec("core"),) * (n_params + n_outs),
                  out_specs=(PartitionSpec("core"),) * n_outs,
                  check_rep=False),
        donate_argnums=donate, keep_unused=True)

    def run(in_maps):
        concat_in = [
            _np.concatenate([_np.asarray(m[name]) for m in in_maps], axis=0)
            for name in in_names]
        concat_zeros = [
            _np.zeros((N_CORES * sh[0], *sh[1:]), dtype)
            for sh, dtype in zero_shapes]
        out_arrs = sharded(*concat_in, *concat_zeros)
        return [
            {name: _np.asarray(out_arrs[i]).reshape(
                N_CORES, *zero_shapes[i][0])[c]
             for i, name in enumerate(out_names)}
            for c in range(N_CORES)]

    _CACHE["runner"] = run
    return run


def _fp8():
    return (ml_dtypes.float8_e4m3fn if hasattr(ml_dtypes, 'float8_e4m3fn')
            else ml_dtypes.float8_e4m3)


def kernel(x, mask, W_qkv, b_qkv, W_out, b_out):
    bf = ml_dtypes.bfloat16
    f8 = _fp8()
    x = np.asarray(x, dtype=np.float32)
    W_qkv = np.asarray(W_qkv, dtype=np.float32)
    b_qkv = np.asarray(b_qkv, dtype=np.float32)
    W_out = np.asarray(W_out, dtype=np.float32)
    b_out = np.asarray(b_out, dtype=np.float32)
    # mask is the causal tril mask (per problem spec); causality is
    # implemented structurally on-device.

    run = _get_runner()

    def pack(wslice):
        # [C, DH] -> [P, 2, KO, P] with C = ko*P + p, DH = s*P + d
        return np.ascontiguousarray(
            wslice.reshape(KO, P, 2, P).transpose(1, 2, 0, 3))

    def w_hilo(wslice):
        # hi/lo fp8 split: 32W ~= Wh + (Wl term via xh) with W1 for xl term
        w32 = wslice * WSCALE
        wh = w32.astype(f8)
        wl = (w32 - wh.astype(np.float32)).astype(f8)
        w1 = wslice.astype(f8)
        return (pack(wh.astype(np.float32)).astype(f8),
                pack(w1.astype(np.float32)).astype(f8),
                pack(wl.astype(np.float32)).astype(f8))

    def pack_b(bslice):
        # [DH] -> [P, 2] with idx = s*P + p, x32
        return np.ascontiguousarray(
            (bslice * WSCALE).reshape(2, P).T).astype(np.float32)

    tri = np.triu(np.ones((P, P), dtype=np.float32))
    tri2 = np.ascontiguousarray(
        np.broadcast_to(tri[:, None, :], (P, 2, P))).astype(bf)
    ident = np.eye(P, dtype=np.float32).astype(bf)

    xhs, xls = [], []
    for b in range(B):
        xt = np.ascontiguousarray(x[b].T)  # [C, T]
        xh = xt.astype(f8)
        xl = ((xt - xh.astype(np.float32)) * WSCALE).astype(f8)
        xhs.append(xh)
        xls.append(xl)

    in_maps = []
    for core in range(N_CORES):
        b, g = divmod(core, GROUPS)
        cs = slice(g * DH, (g + 1) * DH)
        wq3 = w_hilo(W_qkv[:, cs])
        wk3 = w_hilo(W_qkv[:, C:][:, cs])
        wv3 = w_hilo(W_qkv[:, 2 * C:][:, cs])
        in_maps.append({
            "xh": xhs[b], "xl": xls[b],
            "wqh": wq3[0], "wq1": wq3[1], "wql": wq3[2],
            "wkh": wk3[0], "wk1": wk3[1], "wkl": wk3[2],
            "wvh": wv3[0], "wv1": wv3[1], "wvl": wv3[2],
            "wo": np.ascontiguousarray(
                W_out[cs, :].reshape(2, P, C).transpose(1, 0, 2)).astype(bf),
            "bq": pack_b(b_qkv[cs]),
            "bk": pack_b(b_qkv[C:][cs]),
            "bv": np.ascontiguousarray(
                (b_qkv[2 * C:][cs] * WSCALE)[None, :]).astype(np.float32),
            "tri": tri2,
            "ident": ident,
        })

    results = run(in_maps)

    out = np.zeros((B, T, C), dtype=np.float32)
    for core in range(N_CORES):
        b = core // GROUPS
        out[b] += results[core]["out"].astype(np.float32)
    out += b_out[None, None, :]
    return out
